# revision 6
# baseline (speedup 1.0000x reference)
"""Causal self-attention with RoPE on 8 Trainium2 NeuronCores.

Sharding: tensor-parallel over heads (16 heads -> 2 per core). Each core
computes QKV+RoPE+SDPA for its 2 heads over the full batch/sequence, then
the per-head context vectors are AllGathered so every core computes a
256-column slice of the output projection (column-parallel out-proj).

x is uploaded token-sharded (1024 tokens per core) and AllGathered on
device to avoid replicating the 67MB activation 8x over the host link.

All matmuls run as float32r (full PE rate); accumulation stays fp32 in
PSUM. Softmax skips the max-subtraction (scores are ~N(0,1), no overflow
risk) and folds the 1/sqrt(HD) scale into the exp activation. The softmax
is computed in transposed orientation S_T[sk, sq] so no PE transposes are
needed anywhere: the denominator comes from a ones-vector matmul and the
normalization is applied with a partition-broadcast of 1/l.

RoPE's rotate_half is a partition swap in the [hd, s] layout; it is done
with two SBUF->SBUF DMA half-copies plus a sign-folded sin table.
"""

import math

import numpy as np

import concourse.bass as bass
import concourse.mybir as mybir
from concourse.bass_utils import run_bass_kernel_spmd
from concourse.tile import TileContext

B, S, D, H = 4, 2048, 2048, 16
HD = 128  # head dim
NC = 8  # cores
HPC = H // NC  # heads per core = 2
TOK = B * S  # 8192
TOKC = TOK // NC  # tokens uploaded per core = 1024
CH = 256  # phase-1 token chunk
KT = D // 128  # 16 contraction tiles
F32 = mybir.dt.float32
F32R = mybir.dt.float32r
EXP_SCALE = 1.0 / math.sqrt(HD)


def _split_excess_waits(nc):
    """walrus caps sync waits at 1 per instruction (2 for EventSemaphore);
    Tile's kernel-tail drain can carry one wait per outstanding DMA-queue
    semaphore. Hoist the extras onto wait-only NoOps inserted before it."""
    for bb in nc.main_func.blocks:
        insts = bb.instructions
        i = 0
        while i < len(insts):
            ins = insts[i]
            si = getattr(ins, "sync_info", None)
            cap = 2 if isinstance(ins, mybir.InstEventSemaphore) else 1
            if si is not None and len(si.on_wait) > cap:
                extra, keep = si.on_wait[:-cap], si.on_wait[-cap:]
                for k, w in enumerate(extra):
                    nop = mybir.InstNoOp(
                        name=f"{ins.name}_waitsplit_{k}",
                        sync_info=mybir.SyncInfo(on_wait=[w], on_update=[]),
                        engine=ins.engine,
                        bass_nofuse=True,
                    )
                    insts.insert(i, nop)
                    i += 1
                ins.sync_info = mybir.SyncInfo(on_wait=keep, on_update=si.on_update)
            i += 1


def _build():
    nc = bass.Bass()

    # ---- I/O ----
    xTs = nc.dram_tensor("xTs", [D, TOKC], F32, kind="ExternalInput")
    wqk = nc.dram_tensor("wqk", [D, 512], F32, kind="ExternalInput")
    wv = nc.dram_tensor("wv", [D, 256], F32, kind="ExternalInput")
    wout = nc.dram_tensor("wout", [D, 256], F32, kind="ExternalInput")
    copT = nc.dram_tensor("copT", [HD, S], F32, kind="ExternalInput")
    sipT = nc.dram_tensor("sipT", [HD, S], F32, kind="ExternalInput")
    triu = nc.dram_tensor("triu", [128, 128], F32, kind="ExternalInput")
    onesv = nc.dram_tensor("onesv", [128, 1], F32, kind="ExternalInput")

    outp = nc.dram_tensor("outp", [TOK, 256], F32, kind="ExternalOutput")
    ko = nc.dram_tensor("ko", [B, HPC, HD, S], F32, kind="ExternalOutput")
    vo = nc.dram_tensor("vo", [B, S, HPC * HD], F32, kind="ExternalOutput")

    # ---- internal DRAM for collectives ----
    xbounce = nc.dram_tensor("xbounce", [D, TOKC], F32)
    xg = nc.dram_tensor("xg", [NC, D, TOKC], F32, addr_space="Shared")
    ctxb = nc.dram_tensor("ctxb", [B, HPC * HD, S], F32)
    ctxg = nc.dram_tensor("ctxg", [B, NC, HPC * HD, S], F32, addr_space="Shared")

    rr = lambda ap: ap.rearrange("(o p) f -> p o f", p=128)

    from contextlib import ExitStack

    with TileContext(nc) as tc, ExitStack() as ctx:
        cpool = ctx.enter_context(tc.tile_pool(name="const", bufs=1))
        qkpool = ctx.enter_context(tc.tile_pool(name="qk", bufs=1))
        vpool = ctx.enter_context(tc.tile_pool(name="vres", bufs=1))
        stream = ctx.enter_context(tc.tile_pool(name="stream", bufs=2))
        rope = ctx.enter_context(tc.tile_pool(name="rope", bufs=3))
        expp = ctx.enter_context(tc.tile_pool(name="expp", bufs=3))
        lpool = ctx.enter_context(tc.tile_pool(name="lpool", bufs=2))
        cstg = ctx.enter_context(tc.tile_pool(name="cstg", bufs=3))
        octx = ctx.enter_context(tc.tile_pool(name="octx", bufs=6))
        obuf = ctx.enter_context(tc.tile_pool(name="obuf", bufs=3))
        ps_mm = ctx.enter_context(tc.tile_pool(name="ps_mm", bufs=2, space="PSUM"))
        ps_st = ctx.enter_context(tc.tile_pool(name="ps_st", bufs=2, space="PSUM"))
        ps_cx = ctx.enter_context(tc.tile_pool(name="ps_cx", bufs=2, space="PSUM"))
        ps_l = ctx.enter_context(tc.tile_pool(name="ps_l", bufs=2, space="PSUM"))
        dscr = ctx.enter_context(tc.tile_pool(name="dscr", bufs=2, space="DRAM"))

        # ---- constants / weights (resident) ----
        wqk_t = cpool.tile([128, KT, 512], F32R, tag="wqk")
        nc.sync.dma_start(wqk_t[:], rr(wqk[:]).bitcast(F32R))
        wv_t = cpool.tile([128, KT, 256], F32R, tag="wv")
        nc.sync.dma_start(wv_t[:], rr(wv[:]).bitcast(F32R))
        wout_t = cpool.tile([128, KT, 256], F32R, tag="wout")
        nc.sync.dma_start(wout_t[:], rr(wout[:]).bitcast(F32R))
        cop_t = cpool.tile([HD, S], F32, tag="cop")
        nc.sync.dma_start(cop_t[:], copT[:])
        sip_t = cpool.tile([HD, S], F32, tag="sip")
        nc.sync.dma_start(sip_t[:], sipT[:])
        triu_t = cpool.tile([128, 128], F32R, tag="triu")
        nc.sync.dma_start(triu_t[:], triu[:].bitcast(F32R))
        ones_t = cpool.tile([128, 1], F32R, tag="ones")
        nc.sync.dma_start(ones_t[:], onesv[:].bitcast(F32R))

        # ---- x AllGather ----
        nc.sync.dma_start(xbounce[:], xTs[:])
        nc.gpsimd.collective_compute(
            "AllGather",
            mybir.AluOpType.bypass,
            replica_groups=[list(range(NC))],
            ins=[xbounce[:]],
            outs=[xg[:]],
        )

        # resident per-batch tensors (serially reused across batches)
        q_t = [qkpool.tile([HD, S], F32R, tag=f"q{h}", name=f"q{h}") for h in range(HPC)]
        k_t = [qkpool.tile([HD, S], F32R, tag=f"k{h}", name=f"k{h}") for h in range(HPC)]
        v_t = vpool.tile([128, S // 128, 256], F32R, tag="v")

        for b in range(B):
            # ================= phase 1: QKV + RoPE =================
            for t in range(S // CH):
                g0 = b * S + t * CH
                cc, tl = g0 // TOKC, g0 % TOKC
                xc = stream.tile([128, KT, CH], F32R, tag="xc")
                nc.sync.dma_start(
                    xc[:],
                    xg[cc][:, tl : tl + CH].rearrange("(o p) t -> p o t", p=128).bitcast(F32R),
                )
                s_sl = slice(t * CH, (t + 1) * CH)
                # q/k heads: psum = w_tile.T @ x  -> [feat 128, CH] transposed layout
                for fo in range(2 * HPC):
                    ps = ps_mm.tile([128, 512], F32, tag="mm")
                    for k in range(KT):
                        nc.tensor.matmul(
                            ps[:, :CH],
                            wqk_t[:, k, fo * 128 : (fo + 1) * 128],
                            xc[:, k],
                            start=(k == 0),
                            stop=(k == KT - 1),
                        )
                    raw = rope.tile([128, CH], F32, tag="raw")
                    nc.scalar.copy(raw[:], ps[:, :CH])
                    rot = rope.tile([128, CH], F32, tag="rot")
                    nc.sync.dma_start(rot[0:64], raw[64:128])
                    nc.sync.dma_start(rot[64:128], raw[0:64])
                    nc.vector.tensor_mul(rot[:], rot[:], sip_t[:, s_sl])
                    nc.vector.tensor_mul(raw[:], raw[:], cop_t[:, s_sl])
                    if fo < HPC:  # q head
                        nc.vector.tensor_add(q_t[fo][:, s_sl], raw[:], rot[:])
                    else:  # k head: keep fp32 for the ko output, cast copy for SDPA
                        lh = fo - HPC
                        nc.vector.tensor_add(raw[:], raw[:], rot[:])
                        nc.vector.tensor_copy(k_t[lh][:, s_sl], raw[:])
                        nc.sync.dma_start(ko[b, lh, :, s_sl], raw[:])
                # v: psum = x_tile.T @ wv -> [tok 128, 256] natural layout
                for tb in range(CH // 128):
                    ps = ps_mm.tile([128, 512], F32, tag="mm")
                    for k in range(KT):
                        nc.tensor.matmul(
                            ps[:, :256],
                            xc[:, k, tb * 128 : (tb + 1) * 128],
                            wv_t[:, k],
                            start=(k == 0),
                            stop=(k == KT - 1),
                        )
                    vtmp = rope.tile([128, 256], F32, tag="vtmp")
                    nc.scalar.copy(vtmp[:], ps[:, :256])
                    tok0 = t * CH + tb * 128
                    nc.sync.dma_start(vo[b, tok0 : tok0 + 128, :], vtmp[:])
                    nc.vector.tensor_copy(v_t[:, tok0 // 128, :], vtmp[:])

            # ================= phase 2: causal SDPA =================
            for h in range(HPC):
                for sqt in range(S // 512):
                    ctx_ps = ps_cx.tile([128, 512], F32, tag="cx")
                    l_ps = ps_l.tile([1, 512], F32, tag="l")
                    nblk = 4 * (sqt + 1)
                    for sk in range(nblk):
                        c0 = max(0, sk * 128 - sqt * 512)
                        st = ps_st.tile([128, 512], F32, tag="st")
                        nc.tensor.matmul(
                            st[:, c0:],
                            k_t[h][:, sk * 128 : (sk + 1) * 128],
                            q_t[h][:, sqt * 512 + c0 : (sqt + 1) * 512],
                            start=True,
                            stop=True,
                        )
                        ex = expp.tile([128, 512], F32R, tag="ex")
                        nc.scalar.activation(
                            ex[:, c0:], st[:, c0:], mybir.ActivationFunctionType.Exp,
                            scale=EXP_SCALE,
                        )
                        if sk * 128 >= sqt * 512:  # diagonal block: causal mask
                            nc.vector.tensor_mul(
                                ex[:, c0 : c0 + 128], ex[:, c0 : c0 + 128], triu_t[:]
                            )
                        nc.tensor.matmul(
                            l_ps[:, c0:], ones_t[:], ex[:, c0:],
                            start=(sk == 0), stop=(sk == nblk - 1),
                        )
                        nc.tensor.matmul(
                            ctx_ps[:, c0:],
                            v_t[:, sk, h * 128 : (h + 1) * 128],
                            ex[:, c0:],
                            start=(sk == 0),
                            stop=(sk == nblk - 1),
                        )
                    linv = lpool.tile([1, 512], F32, tag="linv")
                    nc.vector.reciprocal(linv[:], l_ps[:])
                    lscr = dscr.tile([1, 512], F32, tag="lscr")
                    nc.sync.dma_start(lscr[:], linv[:])
                    lb = lpool.tile([128, 512], F32, tag="lb")
                    nc.sync.dma_start(lb[:], lscr[:].to_broadcast((128, 512)))
                    cs = cstg.tile([128, 512], F32, tag="cs")
                    nc.vector.tensor_mul(cs[:], ctx_ps[:], lb[:])
                    nc.sync.dma_start(
                        ctxb[b, h * 128 : (h + 1) * 128, sqt * 512 : (sqt + 1) * 512],
                        cs[:],
                    )

            # ================= ctx AllGather + out-proj =================
            nc.gpsimd.collective_compute(
                "AllGather",
                mybir.AluOpType.bypass,
                replica_groups=[list(range(NC))],
                ins=[ctxb[b]],
                outs=[ctxg[b]],
            )
            for tb in range(S // 128):
                ps = ps_mm.tile([128, 512], F32, tag="mm")
                idx = 0
                for cc in range(NC):
                    for fh in range(HPC):
                        ct = octx.tile([128, 128], F32R, tag="octx")
                        nc.sync.dma_start(
                            ct[:],
                            ctxg[b, cc, fh * 128 : (fh + 1) * 128,
                                 tb * 128 : (tb + 1) * 128].bitcast(F32R),
                        )
                        nc.tensor.matmul(
                            ps[:, :256],
                            ct[:],
                            wout_t[:, cc * HPC + fh],
                            start=(idx == 0),
                            stop=(idx == NC * HPC - 1),
                        )
                        idx += 1
                ob = obuf.tile([128, 256], F32, tag="ob")
                nc.scalar.copy(ob[:], ps[:, :256])
                tok0 = b * S + tb * 128
                nc.sync.dma_start(outp[tok0 : tok0 + 128, :], ob[:])

    _split_excess_waits(nc)
    return nc


_NC_CACHE = []


def kernel(x, rope_cos, rope_sin, w_qkv, w_out):
    x = np.asarray(x, np.float32)
    rope_cos = np.asarray(rope_cos, np.float32)
    rope_sin = np.asarray(rope_sin, np.float32)
    w_qkv = np.asarray(w_qkv, np.float32)
    w_out = np.asarray(w_out, np.float32)

    xT = np.ascontiguousarray(x.reshape(TOK, D).T)  # [D, TOK]
    copT = np.ascontiguousarray(rope_cos.T)
    sipT = np.ascontiguousarray(rope_sin.T)
    sipT[:64] = -sipT[:64]  # fold rotate_half sign into the sin table
    triu = np.triu(np.ones((128, 128), np.float32))
    onesv = np.ones((128, 1), np.float32)

    in_maps = []
    for c in range(NC):
        in_maps.append(
            {
                "xTs": np.ascontiguousarray(xT[:, c * TOKC : (c + 1) * TOKC]),
                "wqk": np.ascontiguousarray(
                    np.concatenate(
                        [
                            w_qkv[:, 256 * c : 256 * c + 256],
                            w_qkv[:, D + 256 * c : D + 256 * c + 256],
                        ],
                        axis=1,
                    )
                ),
                "wv": np.ascontiguousarray(
                    w_qkv[:, 2 * D + 256 * c : 2 * D + 256 * c + 256]
                ),
                "wout": np.ascontiguousarray(w_out[:, 256 * c : 256 * c + 256]),
                "copT": copT,
                "sipT": sipT,
                "triu": triu,
                "onesv": onesv,
            }
        )

    if not _NC_CACHE:
        _NC_CACHE.append(_build())
    nc = _NC_CACHE[0]
    res = run_bass_kernel_spmd(nc, in_maps, list(range(NC))).results

    out = np.concatenate([res[c]["outp"] for c in range(NC)], axis=1)
    out = out.reshape(B, S, D)
    k = np.empty((B, H, S, HD), np.float32)
    v = np.empty((B, H, S, HD), np.float32)
    for c in range(NC):
        k[:, HPC * c : HPC * (c + 1)] = res[c]["ko"].transpose(0, 1, 3, 2)
        v[:, HPC * c : HPC * (c + 1)] = (
            res[c]["vo"].reshape(B, S, HPC, HD).transpose(0, 2, 1, 3)
        )
    return out, k, v


# revision 13
# speedup vs baseline: 55.6130x; 55.6130x over previous
"""Causal self-attention with RoPE on 8 Trainium2 NeuronCores.

Sharding: tensor-parallel over heads (16 heads -> 2 per core). Each core
computes QKV+RoPE+SDPA for its 2 heads over the full batch/sequence, then
the per-head context vectors are AllGathered so every core computes a
256-column slice of the output projection (column-parallel out-proj).

x is uploaded token-sharded (1024 tokens per core) and AllGathered on
device to avoid replicating the 67MB activation 8x over the host link.

All matmuls run as float32r (full PE rate); accumulation stays fp32 in
PSUM. Softmax skips the max-subtraction (scores are ~N(0,1), no overflow
risk) and folds the 1/sqrt(HD) scale into the exp activation. The softmax
is computed in transposed orientation S_T[sk, sq] so no PE transposes are
needed anywhere: the denominator comes from a ones-vector matmul and the
normalization is applied with a partition-broadcast of 1/l.

RoPE's rotate_half is a partition swap in the [hd, s] layout; it is done
with two SBUF->SBUF DMA half-copies plus a sign-folded sin table.
"""

import math

import numpy as np

import concourse.bass as bass
import concourse.mybir as mybir
from concourse.bass_utils import run_bass_kernel_spmd
from concourse.tile import TileContext

B, S, D, H = 4, 2048, 2048, 16
HD = 128  # head dim
NC = 8  # cores
HPC = H // NC  # heads per core = 2
TOK = B * S  # 8192
TOKC = TOK // NC  # tokens uploaded per core = 1024
CH = 256  # phase-1 token chunk
KT = D // 128  # 16 contraction tiles
F32 = mybir.dt.float32
F32R = mybir.dt.float32r
EXP_SCALE = 1.0 / math.sqrt(HD)


def _split_excess_waits(nc):
    """walrus caps sync waits at 1 per instruction (2 for EventSemaphore);
    Tile's kernel-tail drain can carry one wait per outstanding DMA-queue
    semaphore. Hoist the extras onto wait-only NoOps inserted before it."""
    for bb in nc.main_func.blocks:
        insts = bb.instructions
        i = 0
        while i < len(insts):
            ins = insts[i]
            si = getattr(ins, "sync_info", None)
            cap = 2 if isinstance(ins, mybir.InstEventSemaphore) else 1
            if si is not None and len(si.on_wait) > cap:
                extra, keep = si.on_wait[:-cap], si.on_wait[-cap:]
                for k, w in enumerate(extra):
                    nop = mybir.InstNoOp(
                        name=f"{ins.name}_waitsplit_{k}",
                        sync_info=mybir.SyncInfo(on_wait=[w], on_update=[]),
                        engine=ins.engine,
                        bass_nofuse=True,
                    )
                    insts.insert(i, nop)
                    i += 1
                ins.sync_info = mybir.SyncInfo(on_wait=keep, on_update=si.on_update)
            i += 1


def _build(with_cc=True, reps=1):
    nc = bass.Bass()

    # ---- I/O ----
    xTs = nc.dram_tensor("xTs", [D, TOKC], F32, kind="ExternalInput")
    wqk = nc.dram_tensor("wqk", [D, 512], F32, kind="ExternalInput")
    wv = nc.dram_tensor("wv", [D, 256], F32, kind="ExternalInput")
    wout = nc.dram_tensor("wout", [D, 256], F32, kind="ExternalInput")
    copT = nc.dram_tensor("copT", [HD, S], F32, kind="ExternalInput")
    sipT = nc.dram_tensor("sipT", [HD, S], F32, kind="ExternalInput")
    triu = nc.dram_tensor("triu", [128, 128], F32, kind="ExternalInput")
    onesv = nc.dram_tensor("onesv", [128, 1], F32, kind="ExternalInput")
    onesr = nc.dram_tensor("onesr", [1, 128], F32, kind="ExternalInput")

    outp = nc.dram_tensor("outp", [256, TOK], F32, kind="ExternalOutput")
    ko = nc.dram_tensor("ko", [B, HPC, HD, S], F32, kind="ExternalOutput")
    vo = nc.dram_tensor("vo", [B, S, HPC * HD], F32, kind="ExternalOutput")

    # ---- internal DRAM for collectives ----
    xbounce = nc.dram_tensor("xbounce", [D, TOKC], F32)
    xg = nc.dram_tensor("xg", [NC, D, TOKC], F32, addr_space="Shared")
    ctxb = nc.dram_tensor("ctxb", [B, HPC * HD, S], F32)
    ctxg = nc.dram_tensor("ctxg", [B, NC, HPC * HD, S], F32, addr_space="Shared")

    rr = lambda ap: ap.rearrange("(o p) f -> p o f", p=128)

    from contextlib import ExitStack

    with TileContext(nc) as tc, ExitStack() as ctx:
        cpool = ctx.enter_context(tc.tile_pool(name="const", bufs=1))
        qkpool = ctx.enter_context(tc.tile_pool(name="qk", bufs=1))
        vpool = ctx.enter_context(tc.tile_pool(name="vres", bufs=1))
        stream = ctx.enter_context(tc.tile_pool(name="stream", bufs=2))
        rope = ctx.enter_context(tc.tile_pool(name="rope", bufs=2))
        expp = ctx.enter_context(tc.tile_pool(name="expp", bufs=3))
        lpool = ctx.enter_context(tc.tile_pool(name="lpool", bufs=2))
        cstg = ctx.enter_context(tc.tile_pool(name="cstg", bufs=2))
        octx = ctx.enter_context(tc.tile_pool(name="octx", bufs=3))
        obuf = ctx.enter_context(tc.tile_pool(name="obuf", bufs=2))
        ps_mm = ctx.enter_context(tc.tile_pool(name="ps_mm", bufs=2, space="PSUM"))
        ps_st = ctx.enter_context(tc.tile_pool(name="ps_st", bufs=2, space="PSUM"))
        ps_cx = ctx.enter_context(tc.tile_pool(name="ps_cx", bufs=2, space="PSUM"))
        ps_l = ctx.enter_context(tc.tile_pool(name="ps_l", bufs=2, space="PSUM"))

        # ---- constants / weights (resident) ----
        wqk_t = cpool.tile([128, KT, 512], F32R, tag="wqk")
        nc.sync.dma_start(wqk_t[:], rr(wqk[:]).bitcast(F32R))
        wv_t = cpool.tile([128, KT, 256], F32R, tag="wv")
        nc.sync.dma_start(wv_t[:], rr(wv[:]).bitcast(F32R))
        wout_t = cpool.tile([128, KT, 256], F32R, tag="wout")
        nc.sync.dma_start(wout_t[:], rr(wout[:]).bitcast(F32R))
        cop_t = cpool.tile([HD, S], F32, tag="cop")
        nc.sync.dma_start(cop_t[:], copT[:])
        sip_t = cpool.tile([HD, S], F32, tag="sip")
        nc.sync.dma_start(sip_t[:], sipT[:])
        triu_t = cpool.tile([128, 128], F32R, tag="triu")
        nc.sync.dma_start(triu_t[:], triu[:].bitcast(F32R))
        ones_t = cpool.tile([128, 1], F32R, tag="ones")
        nc.sync.dma_start(ones_t[:], onesv[:].bitcast(F32R))
        onesr_t = cpool.tile([1, 128], F32R, tag="onesr")
        nc.sync.dma_start(onesr_t[:], onesr[:].bitcast(F32R))

        # ---- x AllGather ----
        nc.sync.dma_start(xbounce[:], xTs[:])
        if with_cc:
            nc.gpsimd.collective_compute(
                "AllGather",
                mybir.AluOpType.bypass,
                replica_groups=[list(range(NC))],
                ins=[xbounce[:]],
                outs=[xg[:]],
            )

        # resident per-batch tensors (serially reused across batches)
        q_t = [qkpool.tile([HD, S], F32R, tag=f"q{h}", name=f"q{h}") for h in range(HPC)]
        k_t = [qkpool.tile([HD, S], F32R, tag=f"k{h}", name=f"k{h}") for h in range(HPC)]
        v_t = vpool.tile([128, S // 128, 256], F32R, tag="v")

        for b in [bb for _ in range(reps) for bb in range(B)]:
            # ================= phase 1: QKV + RoPE =================
            for t in range(S // CH):
                g0 = b * S + t * CH
                cc, tl = g0 // TOKC, g0 % TOKC
                xc = stream.tile([128, KT, CH], F32R, tag="xc")
                nc.sync.dma_start(
                    xc[:],
                    xg[cc][:, tl : tl + CH].rearrange("(o p) t -> p o t", p=128).bitcast(F32R),
                )
                s_sl = slice(t * CH, (t + 1) * CH)
                # q/k heads: psum = w_tile.T @ x  -> [feat 128, CH] transposed layout
                for fo in range(2 * HPC):
                    ps = ps_mm.tile([128, 512], F32, tag="mm")
                    for k in range(KT):
                        nc.tensor.matmul(
                            ps[:, :CH],
                            wqk_t[:, k, fo * 128 : (fo + 1) * 128],
                            xc[:, k],
                            start=(k == 0),
                            stop=(k == KT - 1),
                        )
                    raw = rope.tile([128, CH], F32, tag="raw")
                    nc.scalar.copy(raw[:], ps[:, :CH])
                    rot = rope.tile([128, CH], F32, tag="rot")
                    nc.sync.dma_start(rot[0:64], raw[64:128])
                    nc.sync.dma_start(rot[64:128], raw[0:64])
                    nc.vector.tensor_mul(rot[:], rot[:], sip_t[:, s_sl])
                    nc.vector.tensor_mul(raw[:], raw[:], cop_t[:, s_sl])
                    if fo < HPC:  # q head
                        nc.vector.tensor_add(q_t[fo][:, s_sl], raw[:], rot[:])
                    else:  # k head: keep fp32 for the ko output, cast copy for SDPA
                        lh = fo - HPC
                        nc.vector.tensor_add(raw[:], raw[:], rot[:])
                        nc.vector.tensor_copy(k_t[lh][:, s_sl], raw[:])
                        nc.sync.dma_start(ko[b, lh, :, s_sl], raw[:])
                # v: psum = x_tile.T @ wv -> [tok 128, 256] natural layout
                for tb in range(CH // 128):
                    ps = ps_mm.tile([128, 512], F32, tag="mm")
                    for k in range(KT):
                        nc.tensor.matmul(
                            ps[:, :256],
                            xc[:, k, tb * 128 : (tb + 1) * 128],
                            wv_t[:, k],
                            start=(k == 0),
                            stop=(k == KT - 1),
                        )
                    tok0 = t * CH + tb * 128
                    nc.vector.tensor_copy(v_t[:, tok0 // 128, :], ps[:, :256])
                    nc.sync.dma_start(
                        vo[b, tok0 : tok0 + 128, :],
                        v_t[:, tok0 // 128, :].bitcast(F32),
                    )

            # ================= phase 2: causal SDPA =================
            for h in range(HPC):
                for sqt in range(S // 512):
                    ctx_ps = ps_cx.tile([128, 512], F32, tag="cx")
                    l_ps = ps_l.tile([1, 512], F32, tag="l")
                    nblk = 4 * (sqt + 1)
                    for sk in range(nblk):
                        c0 = max(0, sk * 128 - sqt * 512)
                        st = ps_st.tile([128, 512], F32, tag="st")
                        nc.tensor.matmul(
                            st[:, c0:],
                            k_t[h][:, sk * 128 : (sk + 1) * 128],
                            q_t[h][:, sqt * 512 + c0 : (sqt + 1) * 512],
                            start=True,
                            stop=True,
                        )
                        ex = expp.tile([128, 512], F32R, tag="ex")
                        nc.scalar.activation(
                            ex[:, c0:], st[:, c0:], mybir.ActivationFunctionType.Exp,
                            scale=EXP_SCALE,
                        )
                        if sk * 128 >= sqt * 512:  # diagonal block: causal mask
                            nc.vector.tensor_mul(
                                ex[:, c0 : c0 + 128], ex[:, c0 : c0 + 128], triu_t[:]
                            )
                        nc.tensor.matmul(
                            l_ps[:, c0:], ones_t[:], ex[:, c0:],
                            start=(sk == 0), stop=(sk == nblk - 1),
                        )
                        nc.tensor.matmul(
                            ctx_ps[:, c0:],
                            v_t[:, sk, h * 128 : (h + 1) * 128],
                            ex[:, c0:],
                            start=(sk == 0),
                            stop=(sk == nblk - 1),
                        )
                    linv = lpool.tile([1, 512], F32R, tag="linv")
                    with nc.allow_low_precision(reason="f32r rounding of 1/l"):
                        nc.vector.reciprocal(linv[:], l_ps[:])
                    # broadcast 1/l across partitions with a K=1 outer product
                    lb_ps = ps_st.tile([128, 512], F32, tag="st")
                    nc.tensor.matmul(lb_ps[:], onesr_t[:], linv[:], start=True, stop=True)
                    lb = lpool.tile([128, 512], F32, tag="lb")
                    nc.scalar.copy(lb[:], lb_ps[:])
                    cs = cstg.tile([128, 512], F32, tag="cs")
                    nc.vector.tensor_mul(cs[:], ctx_ps[:], lb[:])
                    nc.sync.dma_start(
                        ctxb[b, h * 128 : (h + 1) * 128, sqt * 512 : (sqt + 1) * 512],
                        cs[:],
                    )

            # ================= ctx AllGather + out-proj =================
            if with_cc:
                nc.gpsimd.collective_compute(
                    "AllGather",
                    mybir.AluOpType.bypass,
                    replica_groups=[list(range(NC))],
                    ins=[ctxb[b]],
                    outs=[ctxg[b]],
                )
            # transposed orientation: psum[col 128, tok 512]; ctx streamed as
            # [128, 512]-token slabs (one DMA per feature tile per window)
            for tw in range(S // 512):
                psT = [
                    ps_mm.tile([128, 512], F32, tag="mm", name=f"o{colt}")
                    for colt in range(2)
                ]
                for f in range(KT):
                    cc, fh = f // HPC, f % HPC
                    slab = octx.tile([128, 512], F32R, tag="octx")
                    nc.sync.dma_start(
                        slab[:],
                        ctxg[b, cc, fh * 128 : (fh + 1) * 128,
                             tw * 512 : (tw + 1) * 512].bitcast(F32R),
                    )
                    for colt in range(2):
                        nc.tensor.matmul(
                            psT[colt][:],
                            wout_t[:, f, colt * 128 : (colt + 1) * 128],
                            slab[:],
                            start=(f == 0),
                            stop=(f == KT - 1),
                        )
                for colt in range(2):
                    ob = obuf.tile([128, 512], F32, tag="ob")
                    nc.scalar.copy(ob[:], psT[colt][:])
                    tok0 = b * S + tw * 512
                    nc.sync.dma_start(
                        outp[colt * 128 : (colt + 1) * 128, tok0 : tok0 + 512],
                        ob[:],
                    )

    _split_excess_waits(nc)
    return nc


_NC_CACHE = []


def kernel(x, rope_cos, rope_sin, w_qkv, w_out):
    x = np.asarray(x, np.float32)
    rope_cos = np.asarray(rope_cos, np.float32)
    rope_sin = np.asarray(rope_sin, np.float32)
    w_qkv = np.asarray(w_qkv, np.float32)
    w_out = np.asarray(w_out, np.float32)

    xT = np.ascontiguousarray(x.reshape(TOK, D).T)  # [D, TOK]
    copT = np.ascontiguousarray(rope_cos.T)
    sipT = np.ascontiguousarray(rope_sin.T)
    sipT[:64] = -sipT[:64]  # fold rotate_half sign into the sin table
    triu = np.triu(np.ones((128, 128), np.float32))
    onesv = np.ones((128, 1), np.float32)
    onesr = np.ones((1, 128), np.float32)

    in_maps = []
    for c in range(NC):
        in_maps.append(
            {
                "xTs": np.ascontiguousarray(xT[:, c * TOKC : (c + 1) * TOKC]),
                "wqk": np.ascontiguousarray(
                    np.concatenate(
                        [
                            w_qkv[:, 256 * c : 256 * c + 256],
                            w_qkv[:, D + 256 * c : D + 256 * c + 256],
                        ],
                        axis=1,
                    )
                ),
                "wv": np.ascontiguousarray(
                    w_qkv[:, 2 * D + 256 * c : 2 * D + 256 * c + 256]
                ),
                "wout": np.ascontiguousarray(w_out[:, 256 * c : 256 * c + 256]),
                "copT": copT,
                "sipT": sipT,
                "triu": triu,
                "onesv": onesv,
                "onesr": onesr,
            }
        )

    if not _NC_CACHE:
        _NC_CACHE.append(_build())
    nc = _NC_CACHE[0]
    res = run_bass_kernel_spmd(nc, in_maps, list(range(NC))).results

    outT = np.concatenate([res[c]["outp"] for c in range(NC)], axis=0)  # [D, TOK]
    out = np.ascontiguousarray(outT.T).reshape(B, S, D)
    k = np.empty((B, H, S, HD), np.float32)
    v = np.empty((B, H, S, HD), np.float32)
    for c in range(NC):
        k[:, HPC * c : HPC * (c + 1)] = res[c]["ko"].transpose(0, 1, 3, 2)
        v[:, HPC * c : HPC * (c + 1)] = (
            res[c]["vo"].reshape(B, S, HPC, HD).transpose(0, 2, 1, 3)
        )
    return out, k, v


# revision 14
# speedup vs baseline: 61.2821x; 1.1019x over previous
"""Causal self-attention with RoPE on 8 Trainium2 NeuronCores.

Sharding: tensor-parallel over heads (16 heads -> 2 per core). Each core
computes QKV+RoPE+SDPA for its 2 heads over the full batch/sequence, then
the per-head context vectors are AllGathered so every core computes a
256-column slice of the output projection (column-parallel out-proj).

x is uploaded token-sharded (1024 tokens per core) and AllGathered on
device to avoid replicating the 67MB activation 8x over the host link.

All matmuls run as float32r (full PE rate); accumulation stays fp32 in
PSUM. Softmax skips the max-subtraction (scores are ~N(0,1), no overflow
risk) and folds the 1/sqrt(HD) scale into the exp activation. The softmax
is computed in transposed orientation S_T[sk, sq] so no PE transposes are
needed anywhere: the denominator comes from a ones-vector matmul and the
normalization is applied with a partition-broadcast of 1/l.

RoPE's rotate_half is a partition swap in the [hd, s] layout; it is done
with two SBUF->SBUF DMA half-copies plus a sign-folded sin table.
"""

import math

import numpy as np

import concourse.bass as bass
import concourse.mybir as mybir
from concourse.bass_utils import run_bass_kernel_spmd
from concourse.tile import TileContext

B, S, D, H = 4, 2048, 2048, 16
HD = 128  # head dim
NC = 8  # cores
HPC = H // NC  # heads per core = 2
TOK = B * S  # 8192
TOKC = TOK // NC  # tokens uploaded per core = 1024
CH = 256  # phase-1 token chunk
KT = D // 128  # 16 contraction tiles
F32 = mybir.dt.float32
F32R = mybir.dt.float32r
EXP_SCALE = 1.0 / math.sqrt(HD)


def _split_excess_waits(nc):
    """walrus caps sync waits at 1 per instruction (2 for EventSemaphore);
    Tile's kernel-tail drain can carry one wait per outstanding DMA-queue
    semaphore. Hoist the extras onto wait-only NoOps inserted before it."""
    for bb in nc.main_func.blocks:
        insts = bb.instructions
        i = 0
        while i < len(insts):
            ins = insts[i]
            si = getattr(ins, "sync_info", None)
            cap = 2 if isinstance(ins, mybir.InstEventSemaphore) else 1
            if si is not None and len(si.on_wait) > cap:
                extra, keep = si.on_wait[:-cap], si.on_wait[-cap:]
                for k, w in enumerate(extra):
                    nop = mybir.InstNoOp(
                        name=f"{ins.name}_waitsplit_{k}",
                        sync_info=mybir.SyncInfo(on_wait=[w], on_update=[]),
                        engine=ins.engine,
                        bass_nofuse=True,
                    )
                    insts.insert(i, nop)
                    i += 1
                ins.sync_info = mybir.SyncInfo(on_wait=keep, on_update=si.on_update)
            i += 1


def _build(with_cc=True, reps=1):
    nc = bass.Bass()

    # ---- I/O ----
    xTs = nc.dram_tensor("xTs", [D, TOKC], F32, kind="ExternalInput")
    wqk = nc.dram_tensor("wqk", [D, 512], F32, kind="ExternalInput")
    wv = nc.dram_tensor("wv", [D, 256], F32, kind="ExternalInput")
    wout = nc.dram_tensor("wout", [D, 256], F32, kind="ExternalInput")
    copT = nc.dram_tensor("copT", [HD, S], F32, kind="ExternalInput")
    sipT = nc.dram_tensor("sipT", [HD, S], F32, kind="ExternalInput")
    triu = nc.dram_tensor("triu", [128, 128], F32, kind="ExternalInput")
    onesv = nc.dram_tensor("onesv", [128, 1], F32, kind="ExternalInput")
    onesr = nc.dram_tensor("onesr", [1, 128], F32, kind="ExternalInput")

    outp = nc.dram_tensor("outp", [256, TOK], F32, kind="ExternalOutput")
    ko = nc.dram_tensor("ko", [B, HPC, HD, S], F32, kind="ExternalOutput")
    vo = nc.dram_tensor("vo", [B, S, HPC * HD], F32, kind="ExternalOutput")

    # ---- internal DRAM for collectives ----
    xbounce = nc.dram_tensor("xbounce", [D, TOKC], F32)
    xg = nc.dram_tensor("xg", [NC, D, TOKC], F32, addr_space="Shared")
    ctxb = nc.dram_tensor("ctxb", [B, HPC * HD, S], F32)
    ctxg = nc.dram_tensor("ctxg", [B, NC, HPC * HD, S], F32, addr_space="Shared")

    rr = lambda ap: ap.rearrange("(o p) f -> p o f", p=128)

    from contextlib import ExitStack

    with TileContext(nc) as tc, ExitStack() as ctx:
        cpool = ctx.enter_context(tc.tile_pool(name="const", bufs=1))
        qkpool = ctx.enter_context(tc.tile_pool(name="qk", bufs=1))
        vpool = ctx.enter_context(tc.tile_pool(name="vres", bufs=1))
        stream = ctx.enter_context(tc.tile_pool(name="stream", bufs=2))
        rope = ctx.enter_context(tc.tile_pool(name="rope", bufs=2))
        expp = ctx.enter_context(tc.tile_pool(name="expp", bufs=3))
        lpool = ctx.enter_context(tc.tile_pool(name="lpool", bufs=2))
        cstg = ctx.enter_context(tc.tile_pool(name="cstg", bufs=2))
        octx = ctx.enter_context(tc.tile_pool(name="octx", bufs=3))
        obuf = ctx.enter_context(tc.tile_pool(name="obuf", bufs=2))
        ps_mm = ctx.enter_context(tc.tile_pool(name="ps_mm", bufs=2, space="PSUM"))
        ps_st = ctx.enter_context(tc.tile_pool(name="ps_st", bufs=2, space="PSUM"))
        ps_cx = ctx.enter_context(tc.tile_pool(name="ps_cx", bufs=2, space="PSUM"))
        ps_l = ctx.enter_context(tc.tile_pool(name="ps_l", bufs=2, space="PSUM"))

        # ---- constants / weights (resident) ----
        wqk_t = cpool.tile([128, KT, 512], F32R, tag="wqk")
        nc.sync.dma_start(wqk_t[:], rr(wqk[:]).bitcast(F32R))
        wv_t = cpool.tile([128, KT, 256], F32R, tag="wv")
        nc.sync.dma_start(wv_t[:], rr(wv[:]).bitcast(F32R))
        wout_t = cpool.tile([128, KT, 256], F32R, tag="wout")
        nc.sync.dma_start(wout_t[:], rr(wout[:]).bitcast(F32R))
        cop_t = cpool.tile([HD, S], F32, tag="cop")
        nc.sync.dma_start(cop_t[:], copT[:])
        sip_t = cpool.tile([HD, S], F32, tag="sip")
        nc.sync.dma_start(sip_t[:], sipT[:])
        triu_t = cpool.tile([128, 128], F32R, tag="triu")
        nc.sync.dma_start(triu_t[:], triu[:].bitcast(F32R))
        ones_t = cpool.tile([128, 1], F32R, tag="ones")
        nc.sync.dma_start(ones_t[:], onesv[:].bitcast(F32R))
        onesr_t = cpool.tile([1, 128], F32R, tag="onesr")
        nc.sync.dma_start(onesr_t[:], onesr[:].bitcast(F32R))

        # ---- x AllGather ----
        nc.sync.dma_start(xbounce[:], xTs[:])
        if with_cc:
            nc.gpsimd.collective_compute(
                "AllGather",
                mybir.AluOpType.bypass,
                replica_groups=[list(range(NC))],
                ins=[xbounce[:]],
                outs=[xg[:]],
            )

        # resident per-batch tensors (serially reused across batches)
        q_t = [qkpool.tile([HD, S], F32R, tag=f"q{h}", name=f"q{h}") for h in range(HPC)]
        k_t = [qkpool.tile([HD, S], F32R, tag=f"k{h}", name=f"k{h}") for h in range(HPC)]
        v_t = vpool.tile([128, S // 128, 256], F32R, tag="v")

        for b in [bb for _ in range(reps) for bb in range(B)]:
            # ================= phase 1: QKV + RoPE =================
            for t in range(S // CH):
                g0 = b * S + t * CH
                cc, tl = g0 // TOKC, g0 % TOKC
                xc = stream.tile([128, KT, CH], F32R, tag="xc")
                nc.sync.dma_start(
                    xc[:],
                    xg[cc][:, tl : tl + CH].rearrange("(o p) t -> p o t", p=128).bitcast(F32R),
                )
                s_sl = slice(t * CH, (t + 1) * CH)
                # q/k heads: psum = w_tile.T @ x  -> [feat 128, CH] transposed layout
                for fo in range(2 * HPC):
                    ps = ps_mm.tile([128, 512], F32, tag="mm")
                    for k in range(KT):
                        nc.tensor.matmul(
                            ps[:, :CH],
                            wqk_t[:, k, fo * 128 : (fo + 1) * 128],
                            xc[:, k],
                            start=(k == 0),
                            stop=(k == KT - 1),
                        )
                    raw = rope.tile([128, CH], F32, tag="raw")
                    nc.scalar.copy(raw[:], ps[:, :CH])
                    rot = rope.tile([128, CH], F32, tag="rot")
                    nc.sync.dma_start(rot[0:64], raw[64:128])
                    nc.sync.dma_start(rot[64:128], raw[0:64])
                    nc.vector.tensor_mul(rot[:], rot[:], sip_t[:, s_sl])
                    nc.vector.tensor_mul(raw[:], raw[:], cop_t[:, s_sl])
                    if fo < HPC:  # q head
                        nc.vector.tensor_add(q_t[fo][:, s_sl], raw[:], rot[:])
                    else:  # k head: keep fp32 for the ko output, cast copy for SDPA
                        lh = fo - HPC
                        nc.vector.tensor_add(raw[:], raw[:], rot[:])
                        nc.vector.tensor_copy(k_t[lh][:, s_sl], raw[:])
                        nc.sync.dma_start(ko[b, lh, :, s_sl], raw[:])
                # v: psum = x_tile.T @ wv -> [tok 128, 256] natural layout
                for tb in range(CH // 128):
                    ps = ps_mm.tile([128, 512], F32, tag="mm")
                    for k in range(KT):
                        nc.tensor.matmul(
                            ps[:, :256],
                            xc[:, k, tb * 128 : (tb + 1) * 128],
                            wv_t[:, k],
                            start=(k == 0),
                            stop=(k == KT - 1),
                        )
                    tok0 = t * CH + tb * 128
                    nc.vector.tensor_copy(v_t[:, tok0 // 128, :], ps[:, :256])
                    nc.sync.dma_start(
                        vo[b, tok0 : tok0 + 128, :],
                        v_t[:, tok0 // 128, :].bitcast(F32),
                    )

            # ================= phase 2: causal SDPA =================
            for h in range(HPC):
                for sqt in range(S // 512):
                    ctx_ps = ps_cx.tile([128, 512], F32, tag="cx")
                    l_ps = ps_l.tile([1, 512], F32, tag="l")
                    nblk = 4 * (sqt + 1)
                    for sk in range(nblk):
                        c0 = max(0, sk * 128 - sqt * 512)
                        st = ps_st.tile([128, 512], F32, tag="st")
                        nc.tensor.matmul(
                            st[:, c0:],
                            k_t[h][:, sk * 128 : (sk + 1) * 128],
                            q_t[h][:, sqt * 512 + c0 : (sqt + 1) * 512],
                            start=True,
                            stop=True,
                        )
                        ex = expp.tile([128, 512], F32R, tag="ex")
                        nc.scalar.activation(
                            ex[:, c0:], st[:, c0:], mybir.ActivationFunctionType.Exp,
                            scale=EXP_SCALE,
                        )
                        if sk * 128 >= sqt * 512:  # diagonal block: causal mask
                            nc.vector.tensor_mul(
                                ex[:, c0 : c0 + 128], ex[:, c0 : c0 + 128], triu_t[:]
                            )
                        nc.tensor.matmul(
                            l_ps[:, c0:], ones_t[:], ex[:, c0:],
                            start=(sk == 0), stop=(sk == nblk - 1),
                        )
                        nc.tensor.matmul(
                            ctx_ps[:, c0:],
                            v_t[:, sk, h * 128 : (h + 1) * 128],
                            ex[:, c0:],
                            start=(sk == 0),
                            stop=(sk == nblk - 1),
                        )
                    linv = lpool.tile([1, 512], F32R, tag="linv")
                    with nc.allow_low_precision(reason="f32r rounding of 1/l"):
                        nc.vector.reciprocal(linv[:], l_ps[:])
                    # broadcast 1/l across partitions with a K=1 outer product
                    lb_ps = ps_st.tile([128, 512], F32, tag="st")
                    nc.tensor.matmul(lb_ps[:], onesr_t[:], linv[:], start=True, stop=True)
                    lb = lpool.tile([128, 512], F32, tag="lb")
                    nc.scalar.copy(lb[:], lb_ps[:])
                    cs = cstg.tile([128, 512], F32, tag="cs")
                    nc.vector.tensor_mul(cs[:], ctx_ps[:], lb[:])
                    nc.sync.dma_start(
                        ctxb[b, h * 128 : (h + 1) * 128, sqt * 512 : (sqt + 1) * 512],
                        cs[:],
                    )

            # ================= ctx AllGather + out-proj =================
            if with_cc:
                nc.gpsimd.collective_compute(
                    "AllGather",
                    mybir.AluOpType.bypass,
                    replica_groups=[list(range(NC))],
                    ins=[ctxb[b]],
                    outs=[ctxg[b]],
                )
            # transposed orientation: psum[col 128, tok 512]; ctx streamed as
            # [128, 512]-token slabs (one DMA per feature tile per window)
            for tw in range(S // 512):
                psT = [
                    ps_mm.tile([128, 512], F32, tag="mm", name=f"o{colt}")
                    for colt in range(2)
                ]
                for f in range(KT):
                    cc, fh = f // HPC, f % HPC
                    slab = octx.tile([128, 512], F32R, tag="octx")
                    nc.sync.dma_start(
                        slab[:],
                        ctxg[b, cc, fh * 128 : (fh + 1) * 128,
                             tw * 512 : (tw + 1) * 512].bitcast(F32R),
                    )
                    for colt in range(2):
                        nc.tensor.matmul(
                            psT[colt][:],
                            wout_t[:, f, colt * 128 : (colt + 1) * 128],
                            slab[:],
                            start=(f == 0),
                            stop=(f == KT - 1),
                        )
                for colt in range(2):
                    ob = obuf.tile([128, 512], F32, tag="ob")
                    nc.scalar.copy(ob[:], psT[colt][:])
                    tok0 = b * S + tw * 512
                    nc.sync.dma_start(
                        outp[colt * 128 : (colt + 1) * 128, tok0 : tok0 + 512],
                        ob[:],
                    )

    _split_excess_waits(nc)
    return nc


_NC_CACHE = []


def _make_in_maps(inputs):
    x = np.asarray(inputs["x"], np.float32)
    rope_cos = np.asarray(inputs["rope_cos"], np.float32)
    rope_sin = np.asarray(inputs["rope_sin"], np.float32)
    w_qkv = np.asarray(inputs["w_qkv"], np.float32)
    w_out = np.asarray(inputs["w_out"], np.float32)

    xT = np.ascontiguousarray(x.reshape(TOK, D).T)  # [D, TOK]
    copT = np.ascontiguousarray(rope_cos.T)
    sipT = np.ascontiguousarray(rope_sin.T)
    sipT[:64] = -sipT[:64]  # fold rotate_half sign into the sin table
    triu = np.triu(np.ones((128, 128), np.float32))
    onesv = np.ones((128, 1), np.float32)
    onesr = np.ones((1, 128), np.float32)

    in_maps = []
    for c in range(NC):
        in_maps.append(
            {
                "xTs": np.ascontiguousarray(xT[:, c * TOKC : (c + 1) * TOKC]),
                "wqk": np.ascontiguousarray(
                    np.concatenate(
                        [
                            w_qkv[:, 256 * c : 256 * c + 256],
                            w_qkv[:, D + 256 * c : D + 256 * c + 256],
                        ],
                        axis=1,
                    )
                ),
                "wv": np.ascontiguousarray(
                    w_qkv[:, 2 * D + 256 * c : 2 * D + 256 * c + 256]
                ),
                "wout": np.ascontiguousarray(w_out[:, 256 * c : 256 * c + 256]),
                "copT": copT,
                "sipT": sipT,
                "triu": triu,
                "onesv": onesv,
                "onesr": onesr,
            }
        )
    return in_maps


def kernel(x, rope_cos, rope_sin, w_qkv, w_out):
    in_maps = _make_in_maps(
        {
            "x": x,
            "rope_cos": rope_cos,
            "rope_sin": rope_sin,
            "w_qkv": w_qkv,
            "w_out": w_out,
        }
    )
    if not _NC_CACHE:
        _NC_CACHE.append(_build())
    nc = _NC_CACHE[0]
    res = run_bass_kernel_spmd(nc, in_maps, list(range(NC))).results

    outT = np.concatenate([res[c]["outp"] for c in range(NC)], axis=0)  # [D, TOK]
    out = np.ascontiguousarray(outT.T).reshape(B, S, D)
    k = np.empty((B, H, S, HD), np.float32)
    v = np.empty((B, H, S, HD), np.float32)
    for c in range(NC):
        k[:, HPC * c : HPC * (c + 1)] = res[c]["ko"].transpose(0, 1, 3, 2)
        v[:, HPC * c : HPC * (c + 1)] = (
            res[c]["vo"].reshape(B, S, HPC, HD).transpose(0, 2, 1, 3)
        )
    return out, k, v
